# revision 2
# baseline (speedup 1.0000x reference)
"""Trainium2 Bass kernel for nn_CausalAttnBlock (GroupNorm + per-frame spatial
self-attention + residual), SPMD over 8 NeuronCores.

Full inputs in / full outputs out. Sharding: the fused B*T frame axis (32
frames) is split 4-frames-per-core; the [C,C] projection weights are
replicated. GroupNorm(num_groups=1) statistics couple all 16 frames of a
sample, so each core computes partial (sum, sum-of-squares) over its shard and
a tiny AllReduce over each sample's 4 cores produces the global stats.

Math layout notes (per frame, C=256 channels, N=H*W=1024 positions):
  - hn = x*g' + b' with g' = gamma*rstd, b' = beta - mean*g' (per channel)
  - q = Wq hn + bq, k likewise; computed as [c_out, n] tiles (bias is
    per-partition there).
  - V^T = hn^T Wv^T computed directly as [m, c] so no transpose is needed
    later; bv is folded out: since softmax rows sum to 1, the V bias
    contributes exactly +bv to the attention output, so it is merged into
    bo' = bo + Wo bv on the host.
  - S^T = k^T q as [m(keys), n(queries)]; softmax over keys becomes a
    partition-direction sum, done with a ones-vector matmul on the PE; the
    max-subtraction is skipped (|S|/16 < ~1 for this operator's scale, exp is
    exact to ~2ulp there).
  - O = V E^T accumulated over key chunks; the softmax normalization 1/Z is
    applied to O as a column scale (it commutes with the output projection).
  - y = x + Wo O_norm + bo'.
All matmuls run in bf16 (inputs rounded once, fp32 PSUM accumulation).
"""

import numpy as np
import ml_dtypes

import concourse.bass as bass
import concourse.bacc as bacc
import concourse.tile as tile
from concourse import mybir
from concourse.bass_utils import run_bass_kernel_spmd

# Problem shape (hardcoded per harness contract)
B, C, T, H, W = 2, 256, 16, 32, 32
N = H * W                 # 1024 positions per frame
F = B * T                 # 32 frames
NCORES = 8
FPC = F // NCORES         # 4 frames per core
CS = C // 128             # 2 channel subtiles
EPS = 1e-6
CNT = C * T * H * W       # elements per sample for groupnorm stats
BF16 = mybir.dt.bfloat16
F32 = mybir.dt.float32

_CACHE = {}


def build_nc(repeat: int = 1):
    """Build the per-core Bass program (identical on all cores)."""
    nc = bacc.Bacc("TRN2", target_bir_lowering=False, debug=False,
                   num_devices=NCORES)

    xin = nc.dram_tensor("xin", [128, CS, FPC, N], F32, kind="ExternalInput")
    wq = nc.dram_tensor("wq", [128, CS, C], BF16, kind="ExternalInput")
    wk = nc.dram_tensor("wk", [128, CS, C], BF16, kind="ExternalInput")
    wv = nc.dram_tensor("wv", [128, CS, C], BF16, kind="ExternalInput")
    wo = nc.dram_tensor("wo", [128, CS, C], BF16, kind="ExternalInput")
    bqd = nc.dram_tensor("bq", [128, CS], F32, kind="ExternalInput")
    bkd = nc.dram_tensor("bk", [128, CS], F32, kind="ExternalInput")
    bod = nc.dram_tensor("bop", [128, CS], F32, kind="ExternalInput")
    gad = nc.dram_tensor("gamma", [128, CS], F32, kind="ExternalInput")
    bed = nc.dram_tensor("beta", [128, CS], F32, kind="ExternalInput")
    y = nc.dram_tensor("y", [128, CS, FPC, N], F32, kind="ExternalOutput")

    with tile.TileContext(nc) as tc:
        with (
            tc.tile_pool(name="singles", bufs=1) as singles,
            tc.tile_pool(name="frames", bufs=2) as fr,
            tc.tile_pool(name="psmm", bufs=3, space="PSUM") as psmm,
            tc.tile_pool(name="psz", bufs=1, space="PSUM") as psz,
            tc.tile_pool(name="dram", bufs=2, space="DRAM") as dram,
        ):
            # ---- persistent loads ----
            xt = singles.tile([128, CS, FPC, N], F32)
            for s in range(CS):
                for f in range(FPC):
                    nc.sync.dma_start(xt[:, s, f, :], xin[:, s, f, :])

            wqt = singles.tile([128, CS, C], BF16)
            wkt = singles.tile([128, CS, C], BF16)
            wvt = singles.tile([128, CS, C], BF16)
            wot = singles.tile([128, CS, C], BF16)
            for wtile, wdram in ((wqt, wq), (wkt, wk), (wvt, wv), (wot, wo)):
                nc.sync.dma_start(wtile[:], wdram[:])
            bqt = singles.tile([128, CS], F32)
            bkt = singles.tile([128, CS], F32)
            bot = singles.tile([128, CS], F32)
            gat = singles.tile([128, CS], F32)
            bet = singles.tile([128, CS], F32)
            for btile, bdram in ((bqt, bqd), (bkt, bkd), (bot, bod),
                                 (gat, gad), (bet, bed)):
                nc.sync.dma_start(btile[:], bdram[:])

            ones_f = singles.tile([128, 1], F32)
            nc.vector.memset(ones_f[:], 1.0)
            ones_b = singles.tile([128, 1], BF16)
            nc.vector.memset(ones_b[:], 1.0)
            eps_t = singles.tile([128, 1], F32)
            nc.vector.memset(eps_t[:], EPS)

            # ---- groupnorm stats: per-partition mean/var over this shard ----
            nchunk = CS * FPC * (N // 512)  # 16 chunks of 512
            stats = singles.tile([128, nchunk, 6], F32)
            idx = 0
            for s in range(CS):
                for f in range(FPC):
                    for h in range(N // 512):
                        nc.vector.bn_stats(
                            out=stats[:, idx, :],
                            in_=xt[:, s, f, 512 * h:512 * (h + 1)],
                        )
                        idx += 1
            mv = singles.tile([128, 2], F32)
            nc.vector.bn_aggr(out=mv[:], in_=stats[:])

            # partial sums for this shard: S_p = mean*8192, SS_p = (var+mean^2)*8192
            per_part = CS * FPC * N  # 8192 elements per partition
            s2 = singles.tile([128, 2], F32)
            nc.vector.tensor_scalar_mul(s2[:, 0:1], mv[:, 0:1], float(per_part))
            msq = singles.tile([128, 1], F32)
            nc.vector.tensor_mul(msq[:], mv[:, 0:1], mv[:, 0:1])
            nc.vector.tensor_add(msq[:], msq[:], mv[:, 1:2])
            nc.vector.tensor_scalar_mul(s2[:, 1:2], msq[:], float(per_part))

            # partition-sum via ones matmul -> [1, 2]
            pstat = psz.tile([1, 2], F32, tag="z")
            nc.tensor.matmul(pstat[:], ones_f[:], s2[:], start=True, stop=True)
            ar_sb = singles.tile([1, 2], F32)
            nc.any.tensor_copy(out=ar_sb[:], in_=pstat[:])

            # AllReduce within each sample's 4 cores
            arin = dram.tile([1, 2], F32)
            arout = dram.tile([1, 2], F32)
            nc.sync.dma_start(arin[:], ar_sb[:])
            nc.gpsimd.collective_compute(
                "AllReduce", mybir.AluOpType.add,
                replica_groups=[[0, 1, 2, 3], [4, 5, 6, 7]],
                ins=[arin[:].opt()], outs=[arout[:].opt()],
            )
            # broadcast [1,2] -> [128,2] so every partition computes stats
            st_bc = singles.tile([128, 2], F32)
            nc.sync.dma_start(
                st_bc[:],
                bass.AP(tensor=arout[:].tensor, offset=arout[:].offset,
                        ap=[[0, 128], [1, 2]]),
            )
            mean_g = singles.tile([128, 1], F32)
            nc.vector.tensor_scalar_mul(mean_g[:], st_bc[:, 0:1], 1.0 / CNT)
            var_g = singles.tile([128, 1], F32)
            nc.vector.tensor_scalar_mul(var_g[:], st_bc[:, 1:2], 1.0 / CNT)
            mg2 = singles.tile([128, 1], F32)
            nc.vector.tensor_mul(mg2[:], mean_g[:], mean_g[:])
            nc.vector.tensor_tensor(var_g[:], var_g[:], mg2[:],
                                    mybir.AluOpType.subtract)
            # rstd = exp(-0.5*ln(var+eps))  (Ln/Exp share one ACT table set)
            lnv = singles.tile([128, 1], F32)
            nc.scalar.activation(out=lnv[:], in_=var_g[:],
                                 func=mybir.ActivationFunctionType.Ln,
                                 bias=eps_t[:], scale=1.0)
            rstd = singles.tile([128, 1], F32)
            nc.scalar.activation(out=rstd[:], in_=lnv[:],
                                 func=mybir.ActivationFunctionType.Exp,
                                 scale=-0.5)
            # g' = gamma*rstd ; b' = beta - mean*g'
            gp = singles.tile([128, CS], F32)
            nc.vector.tensor_scalar_mul(gp[:], gat[:], rstd[:])
            bp = singles.tile([128, CS], F32)
            nc.vector.tensor_scalar_mul(bp[:], gp[:], mean_g[:])
            nc.vector.tensor_tensor(bp[:], bet[:], bp[:],
                                    mybir.AluOpType.subtract)

            # ---- per-frame attention ----
            for _ in range(repeat):
                for f in range(FPC):
                    # normalized activations, bf16
                    hn = fr.tile([128, CS, N], BF16, tag="hn")
                    for s in range(CS):
                        nc.any.tensor_scalar(
                            out=hn[:, s, :], in0=xt[:, s, f, :],
                            scalar1=gp[:, s:s + 1], scalar2=bp[:, s:s + 1],
                            op0=mybir.AluOpType.mult, op1=mybir.AluOpType.add)

                    # V^T [m, c] = hn^T Wv^T
                    vt = fr.tile([128, 8, C], BF16, tag="vt")
                    for mi in range(8):
                        vps = psmm.tile([128, C], F32, tag="mm")
                        for s in range(CS):
                            nc.tensor.matmul(
                                vps[:], hn[:, s, 128 * mi:128 * (mi + 1)],
                                wvt[:, s, :], start=(s == 0), stop=(s == CS - 1))
                        nc.any.tensor_copy(out=vt[:, mi, :], in_=vps[:])

                    # Q, K  [c_out, n] with bias
                    qt = fr.tile([128, CS, N], BF16, tag="qt")
                    kt = fr.tile([128, CS, N], BF16, tag="kt")
                    for dst, wt, bt in ((qt, wqt, bqt), (kt, wkt, bkt)):
                        for j in range(CS):
                            pps = psmm.tile([128, N], F32, tag="mm")
                            for h in range(2):
                                hs = slice(512 * h, 512 * (h + 1))
                                for s in range(CS):
                                    nc.tensor.matmul(
                                        pps[:, hs],
                                        wt[:, s, 128 * j:128 * (j + 1)],
                                        hn[:, s, hs], start=(s == 0),
                                        stop=(s == CS - 1))
                            nc.any.tensor_scalar(
                                out=dst[:, j, :], in0=pps[:],
                                scalar1=bt[:, j:j + 1], scalar2=None,
                                op0=mybir.AluOpType.add)

                    # S^T chunks + exp -> E^T ; Z column sums on the side
                    et = fr.tile([128, 8, N], BF16, tag="et")
                    zps = psz.tile([1, N], F32, tag="z")
                    for mi in range(8):
                        sps = psmm.tile([128, N], F32, tag="mm")
                        for h in range(2):
                            hs = slice(512 * h, 512 * (h + 1))
                            for s in range(CS):
                                nc.tensor.matmul(
                                    sps[:, hs],
                                    kt[:, s, 128 * mi:128 * (mi + 1)],
                                    qt[:, s, hs], start=(s == 0),
                                    stop=(s == CS - 1))
                        nc.scalar.activation(
                            out=et[:, mi, :], in_=sps[:],
                            func=mybir.ActivationFunctionType.Exp,
                            scale=float(C) ** -0.5)
                        for h in range(2):
                            hs = slice(512 * h, 512 * (h + 1))
                            nc.tensor.matmul(zps[:, hs], ones_b[:],
                                             et[:, mi, hs],
                                             start=(mi == 0), stop=(mi == 7))

                    # R = 1/Z via exp(-ln(Z)); broadcast to 128 partitions
                    lnz = fr.tile([1, N], F32, tag="lnz")
                    nc.scalar.activation(out=lnz[:], in_=zps[:],
                                         func=mybir.ActivationFunctionType.Ln,
                                         scale=1.0)
                    r_sb = fr.tile([1, N], F32, tag="r_sb")
                    nc.scalar.activation(out=r_sb[:], in_=lnz[:],
                                         func=mybir.ActivationFunctionType.Exp,
                                         scale=-1.0)
                    r_dram = dram.tile([1, N], F32)
                    nc.sync.dma_start(r_dram[:], r_sb[:])
                    rb = fr.tile([128, N], F32, tag="rb")
                    nc.sync.dma_start(
                        rb[:],
                        bass.AP(tensor=r_dram[:].tensor,
                                offset=r_dram[:].offset,
                                ap=[[0, 128], [1, N]]),
                    )

                    # O = V E^T, normalized by R on the way to SBUF
                    osb = fr.tile([128, CS, N], BF16, tag="osb")
                    for j in range(CS):
                        ops = psmm.tile([128, N], F32, tag="mm")
                        for h in range(2):
                            hs = slice(512 * h, 512 * (h + 1))
                            for mi in range(8):
                                nc.tensor.matmul(
                                    ops[:, hs],
                                    vt[:, mi, 128 * j:128 * (j + 1)],
                                    et[:, mi, hs], start=(mi == 0),
                                    stop=(mi == 7))
                        nc.any.tensor_tensor(out=osb[:, j, :], in0=ops[:],
                                             in1=rb[:], op=mybir.AluOpType.mult)

                    # P = Wo O_norm ; y = x + P + bo'
                    fin = fr.tile([128, CS, N], F32, tag="fin")
                    for j in range(CS):
                        pps = psmm.tile([128, N], F32, tag="mm")
                        for h in range(2):
                            hs = slice(512 * h, 512 * (h + 1))
                            for s in range(CS):
                                nc.tensor.matmul(
                                    pps[:, hs],
                                    wot[:, s, 128 * j:128 * (j + 1)],
                                    osb[:, s, hs], start=(s == 0),
                                    stop=(s == CS - 1))
                        nc.any.tensor_scalar(
                            out=fin[:, j, :], in0=pps[:],
                            scalar1=bot[:, j:j + 1], scalar2=None,
                            op0=mybir.AluOpType.add)
                        nc.any.tensor_tensor(out=fin[:, j, :], in0=fin[:, j, :],
                                             in1=xt[:, j, f, :],
                                             op=mybir.AluOpType.add)
                        nc.sync.dma_start(y[:, j, f, :], fin[:, j, :])

    nc.compile()
    return nc


def _get_nc(repeat: int = 1):
    if repeat not in _CACHE:
        _CACHE[repeat] = build_nc(repeat)
    return _CACHE[repeat]


def _prep_inputs(x, gamma, beta, wq, bq, wk, bk, wv, bv, wo, bo):
    """Host-side sharding / layout prep -> per-core input maps."""
    bf = ml_dtypes.bfloat16

    def wprep(w):
        # lhsT layout [ci, c_out] striped to [p, cs, c_out]
        return np.ascontiguousarray(
            w.T.reshape(CS, 128, C).transpose(1, 0, 2)).astype(bf)

    def vprep(v):
        # per-channel [C] -> [128, CS]
        return np.ascontiguousarray(v.reshape(CS, 128).T).astype(np.float32)

    wq_h, wk_h, wv_h, wo_h = wprep(wq), wprep(wk), wprep(wv), wprep(wo)
    bop = (wo.astype(np.float64) @ bv.astype(np.float64)).astype(np.float32) + bo
    shared = {
        "wq": wq_h, "wk": wk_h, "wv": wv_h, "wo": wo_h,
        "bq": vprep(bq), "bk": vprep(bk), "bop": vprep(bop),
        "gamma": vprep(gamma), "beta": vprep(beta),
    }

    frames = np.ascontiguousarray(
        x.transpose(0, 2, 1, 3, 4).reshape(F, C, N))  # [32, 256, 1024]
    in_maps = []
    for c in range(NCORES):
        sh = frames[FPC * c:FPC * (c + 1)]           # [4, 256, 1024]
        arr = np.ascontiguousarray(
            sh.transpose(1, 0, 2).reshape(CS, 128, FPC, N).transpose(1, 0, 2, 3))
        in_maps.append({"xin": arr.astype(np.float32), **shared})
    return in_maps


def _assemble(results):
    frames = np.empty((F, C, N), np.float32)
    for c in range(NCORES):
        arr = results[c]["y"]                        # [128, CS, FPC, N]
        frames[FPC * c:FPC * (c + 1)] = (
            arr.transpose(1, 0, 2, 3).reshape(C, FPC, N).transpose(1, 0, 2))
    return frames.reshape(B, T, C, H, W).transpose(0, 2, 1, 3, 4)


def kernel(**inputs):
    inputs = {k: np.asarray(v) for k, v in inputs.items()}
    in_maps = _prep_inputs(**inputs)
    nc = _get_nc()
    res = run_bass_kernel_spmd(nc, in_maps, core_ids=list(range(NCORES)))
    return _assemble(res.results)


def run_repeat(inputs, repeat):
    """For timing: run a variant whose frame loop executes `repeat` times."""
    in_maps = _prep_inputs(**{k: np.asarray(v) for k, v in inputs.items()})
    nc = _get_nc(repeat)
    res = run_bass_kernel_spmd(nc, in_maps, core_ids=list(range(NCORES)))
    return _assemble(res.results)


# revision 4
# speedup vs baseline: 102.2833x; 102.2833x over previous
"""Trainium2 Bass kernel for nn_CausalAttnBlock (GroupNorm + per-frame spatial
self-attention + residual), SPMD over 8 NeuronCores.

Full inputs in / full outputs out. Sharding: the fused B*T frame axis (32
frames) is split 4-frames-per-core; the [C,C] projection weights are
replicated. GroupNorm(num_groups=1) statistics couple all 16 frames of a
sample, so each core computes partial (sum, sum-of-squares) over its shard and
a tiny AllReduce over each sample's 4 cores produces the global stats.

Math layout notes (per frame, C=256 channels, N=H*W=1024 positions):
  - hn = x*g' + b' with g' = gamma*rstd, b' = beta - mean*g' (per channel)
  - q = Wq hn + bq, k likewise; computed as [c_out, n] tiles (bias is
    per-partition there).
  - V^T = hn^T Wv^T computed directly as [m, c] so no transpose is needed
    later; bv is folded out: since softmax rows sum to 1, the V bias
    contributes exactly +bv to the attention output, so it is merged into
    bo' = bo + Wo bv on the host.
  - S^T = k^T q as [m(keys), n(queries)]; softmax over keys becomes a
    partition-direction sum, done with a ones-vector matmul on the PE; the
    max-subtraction is skipped (|S|/16 < ~1 for this operator's scale, exp is
    exact to ~2ulp there).
  - O = V E^T accumulated over key chunks; the softmax normalization 1/Z is
    applied to O as a column scale (it commutes with the output projection).
  - y = x + Wo O_norm + bo'.
All matmuls run in bf16 (inputs rounded once, fp32 PSUM accumulation).
"""

import numpy as np
import ml_dtypes

import jax
import concourse.bass as bass
import concourse.bacc as bacc
import concourse.tile as tile
from concourse import bass2jax, mybir
from jax.experimental.shard_map import shard_map
from jax.sharding import Mesh, PartitionSpec

# Problem shape (hardcoded per harness contract)
B, C, T, H, W = 2, 256, 16, 32, 32
N = H * W                 # 1024 positions per frame
F = B * T                 # 32 frames
NCORES = 8
FPC = F // NCORES         # 4 frames per core
CS = C // 128             # 2 channel subtiles
EPS = 1e-6
CNT = C * T * H * W       # elements per sample for groupnorm stats
BF16 = mybir.dt.bfloat16
F32 = mybir.dt.float32

_CACHE = {}


def build_nc(repeat: int = 1, collective: bool = True):
    """Build the per-core Bass program (identical on all cores)."""
    nc = bacc.Bacc("TRN2", target_bir_lowering=False, debug=False,
                   num_devices=NCORES)

    xin = nc.dram_tensor("xin", [128, CS, FPC, N], F32, kind="ExternalInput")
    wq = nc.dram_tensor("wq", [128, CS, C], BF16, kind="ExternalInput")
    wk = nc.dram_tensor("wk", [128, CS, C], BF16, kind="ExternalInput")
    wv = nc.dram_tensor("wv", [128, CS, C], BF16, kind="ExternalInput")
    wo = nc.dram_tensor("wo", [128, CS, C], BF16, kind="ExternalInput")
    bqd = nc.dram_tensor("bq", [128, CS], F32, kind="ExternalInput")
    bkd = nc.dram_tensor("bk", [128, CS], F32, kind="ExternalInput")
    bod = nc.dram_tensor("bop", [128, CS], F32, kind="ExternalInput")
    gad = nc.dram_tensor("gamma", [128, CS], F32, kind="ExternalInput")
    bed = nc.dram_tensor("beta", [128, CS], F32, kind="ExternalInput")
    y = nc.dram_tensor("y", [128, CS, FPC, N], F32, kind="ExternalOutput")

    with tile.TileContext(nc) as tc:
        with (
            tc.tile_pool(name="singles", bufs=1) as singles,
            tc.tile_pool(name="frames", bufs=2) as fr,
            tc.tile_pool(name="psmm", bufs=3, space="PSUM") as psmm,
            tc.tile_pool(name="psz", bufs=1, space="PSUM") as psz,
            tc.tile_pool(name="dram", bufs=2, space="DRAM") as dram,
        ):
            # ---- persistent loads ----
            xt = singles.tile([128, CS, FPC, N], F32)
            for s in range(CS):
                for f in range(FPC):
                    nc.sync.dma_start(xt[:, s, f, :], xin[:, s, f, :])

            wqt = singles.tile([128, CS, C], BF16)
            wkt = singles.tile([128, CS, C], BF16)
            wvt = singles.tile([128, CS, C], BF16)
            wot = singles.tile([128, CS, C], BF16)
            for wtile, wdram in ((wqt, wq), (wkt, wk), (wvt, wv), (wot, wo)):
                nc.sync.dma_start(wtile[:], wdram[:])
            bqt = singles.tile([128, CS], F32)
            bkt = singles.tile([128, CS], F32)
            bot = singles.tile([128, CS], F32)
            gat = singles.tile([128, CS], F32)
            bet = singles.tile([128, CS], F32)
            for btile, bdram in ((bqt, bqd), (bkt, bkd), (bot, bod),
                                 (gat, gad), (bet, bed)):
                nc.sync.dma_start(btile[:], bdram[:])

            ones_f = singles.tile([128, 1], F32)
            nc.vector.memset(ones_f[:], 1.0)
            ones_b = singles.tile([128, 1], BF16)
            nc.vector.memset(ones_b[:], 1.0)
            eps_t = singles.tile([128, 1], F32)
            nc.vector.memset(eps_t[:], EPS)

            # ---- groupnorm stats: per-partition mean/var over this shard ----
            nchunk = CS * FPC * (N // 512)  # 16 chunks of 512
            stats = singles.tile([128, nchunk, 6], F32)
            idx = 0
            for s in range(CS):
                for f in range(FPC):
                    for h in range(N // 512):
                        nc.vector.bn_stats(
                            out=stats[:, idx, :],
                            in_=xt[:, s, f, 512 * h:512 * (h + 1)],
                        )
                        idx += 1
            mv = singles.tile([128, 2], F32)
            nc.vector.bn_aggr(out=mv[:], in_=stats[:])

            # partial sums for this shard: S_p = mean*8192, SS_p = (var+mean^2)*8192
            per_part = CS * FPC * N  # 8192 elements per partition
            s2 = singles.tile([128, 2], F32)
            nc.vector.tensor_scalar_mul(s2[:, 0:1], mv[:, 0:1], float(per_part))
            msq = singles.tile([128, 1], F32)
            nc.vector.tensor_mul(msq[:], mv[:, 0:1], mv[:, 0:1])
            nc.vector.tensor_add(msq[:], msq[:], mv[:, 1:2])
            nc.vector.tensor_scalar_mul(s2[:, 1:2], msq[:], float(per_part))

            # partition-sum via ones matmul -> [1, 2]
            pstat = psz.tile([1, 2], F32, tag="z")
            nc.tensor.matmul(pstat[:], ones_f[:], s2[:], start=True, stop=True)
            ar_sb = singles.tile([1, 2], F32)
            nc.any.tensor_copy(out=ar_sb[:], in_=pstat[:])

            # AllReduce within each sample's 4 cores
            arin = dram.tile([1, 2], F32)
            arout = dram.tile([1, 2], F32)
            nc.sync.dma_start(arin[:], ar_sb[:])
            if collective:
                nc.gpsimd.collective_compute(
                    "AllReduce", mybir.AluOpType.add,
                    replica_groups=[[0, 1, 2, 3], [4, 5, 6, 7]],
                    ins=[arin[:].opt()], outs=[arout[:].opt()],
                )
            else:
                nc.sync.dma_start(arout[:], arin[:])
            # broadcast [1,2] -> [128,2] so every partition computes stats
            st_bc = singles.tile([128, 2], F32)
            nc.sync.dma_start(
                st_bc[:],
                bass.AP(tensor=arout[:].tensor, offset=arout[:].offset,
                        ap=[[0, 128], [1, 2]]),
            )
            mean_g = singles.tile([128, 1], F32)
            nc.vector.tensor_scalar_mul(mean_g[:], st_bc[:, 0:1], 1.0 / CNT)
            var_g = singles.tile([128, 1], F32)
            nc.vector.tensor_scalar_mul(var_g[:], st_bc[:, 1:2], 1.0 / CNT)
            mg2 = singles.tile([128, 1], F32)
            nc.vector.tensor_mul(mg2[:], mean_g[:], mean_g[:])
            nc.vector.tensor_tensor(var_g[:], var_g[:], mg2[:],
                                    mybir.AluOpType.subtract)
            # rstd = exp(-0.5*ln(var+eps))  (Ln/Exp share one ACT table set)
            lnv = singles.tile([128, 1], F32)
            nc.scalar.activation(out=lnv[:], in_=var_g[:],
                                 func=mybir.ActivationFunctionType.Ln,
                                 bias=eps_t[:], scale=1.0)
            rstd = singles.tile([128, 1], F32)
            nc.scalar.activation(out=rstd[:], in_=lnv[:],
                                 func=mybir.ActivationFunctionType.Exp,
                                 scale=-0.5)
            # g' = gamma*rstd ; b' = beta - mean*g'
            gp = singles.tile([128, CS], F32)
            nc.vector.tensor_scalar_mul(gp[:], gat[:], rstd[:])
            bp = singles.tile([128, CS], F32)
            nc.vector.tensor_scalar_mul(bp[:], gp[:], mean_g[:])
            nc.vector.tensor_tensor(bp[:], bet[:], bp[:],
                                    mybir.AluOpType.subtract)

            # ---- per-frame attention ----
            for _ in range(repeat):
                for f in range(FPC):
                    # normalized activations, bf16
                    hn = fr.tile([128, CS, N], BF16, tag="hn")
                    for s in range(CS):
                        nc.any.tensor_scalar(
                            out=hn[:, s, :], in0=xt[:, s, f, :],
                            scalar1=gp[:, s:s + 1], scalar2=bp[:, s:s + 1],
                            op0=mybir.AluOpType.mult, op1=mybir.AluOpType.add)

                    # V^T [m, c] = hn^T Wv^T
                    vt = fr.tile([128, 8, C], BF16, tag="vt")
                    for mi in range(8):
                        vps = psmm.tile([128, C], F32, tag="mm")
                        for s in range(CS):
                            nc.tensor.matmul(
                                vps[:], hn[:, s, 128 * mi:128 * (mi + 1)],
                                wvt[:, s, :], start=(s == 0), stop=(s == CS - 1))
                        nc.any.tensor_copy(out=vt[:, mi, :], in_=vps[:])

                    # Q, K  [c_out, n] with bias
                    qt = fr.tile([128, CS, N], BF16, tag="qt")
                    kt = fr.tile([128, CS, N], BF16, tag="kt")
                    for dst, wt, bt in ((qt, wqt, bqt), (kt, wkt, bkt)):
                        for j in range(CS):
                            pps = psmm.tile([128, N], F32, tag="mm")
                            for h in range(2):
                                hs = slice(512 * h, 512 * (h + 1))
                                for s in range(CS):
                                    nc.tensor.matmul(
                                        pps[:, hs],
                                        wt[:, s, 128 * j:128 * (j + 1)],
                                        hn[:, s, hs], start=(s == 0),
                                        stop=(s == CS - 1))
                            nc.any.tensor_scalar(
                                out=dst[:, j, :], in0=pps[:],
                                scalar1=bt[:, j:j + 1], scalar2=None,
                                op0=mybir.AluOpType.add)

                    # S^T chunks + exp -> E^T ; Z column sums on the side
                    et = fr.tile([128, 8, N], BF16, tag="et")
                    zps = psz.tile([1, N], F32, tag="z")
                    for mi in range(8):
                        sps = psmm.tile([128, N], F32, tag="mm")
                        for h in range(2):
                            hs = slice(512 * h, 512 * (h + 1))
                            for s in range(CS):
                                nc.tensor.matmul(
                                    sps[:, hs],
                                    kt[:, s, 128 * mi:128 * (mi + 1)],
                                    qt[:, s, hs], start=(s == 0),
                                    stop=(s == CS - 1))
                        nc.scalar.activation(
                            out=et[:, mi, :], in_=sps[:],
                            func=mybir.ActivationFunctionType.Exp,
                            scale=float(C) ** -0.5)
                        for h in range(2):
                            hs = slice(512 * h, 512 * (h + 1))
                            nc.tensor.matmul(zps[:, hs], ones_b[:],
                                             et[:, mi, hs],
                                             start=(mi == 0), stop=(mi == 7))

                    # R = 1/Z via exp(-ln(Z)); broadcast to 128 partitions
                    lnz = fr.tile([1, N], F32, tag="lnz")
                    nc.scalar.activation(out=lnz[:], in_=zps[:],
                                         func=mybir.ActivationFunctionType.Ln,
                                         scale=1.0)
                    r_sb = fr.tile([1, N], F32, tag="r_sb")
                    nc.scalar.activation(out=r_sb[:], in_=lnz[:],
                                         func=mybir.ActivationFunctionType.Exp,
                                         scale=-1.0)
                    r_dram = dram.tile([1, N], F32)
                    nc.sync.dma_start(r_dram[:], r_sb[:])
                    rb = fr.tile([128, N], F32, tag="rb")
                    nc.sync.dma_start(
                        rb[:],
                        bass.AP(tensor=r_dram[:].tensor,
                                offset=r_dram[:].offset,
                                ap=[[0, 128], [1, N]]),
                    )

                    # O = V E^T, normalized by R on the way to SBUF
                    osb = fr.tile([128, CS, N], BF16, tag="osb")
                    for j in range(CS):
                        ops = psmm.tile([128, N], F32, tag="mm")
                        for h in range(2):
                            hs = slice(512 * h, 512 * (h + 1))
                            for mi in range(8):
                                nc.tensor.matmul(
                                    ops[:, hs],
                                    vt[:, mi, 128 * j:128 * (j + 1)],
                                    et[:, mi, hs], start=(mi == 0),
                                    stop=(mi == 7))
                        nc.any.tensor_tensor(out=osb[:, j, :], in0=ops[:],
                                             in1=rb[:], op=mybir.AluOpType.mult)

                    # P = Wo O_norm ; y = x + P + bo'
                    fin = fr.tile([128, CS, N], F32, tag="fin")
                    for j in range(CS):
                        pps = psmm.tile([128, N], F32, tag="mm")
                        for h in range(2):
                            hs = slice(512 * h, 512 * (h + 1))
                            for s in range(CS):
                                nc.tensor.matmul(
                                    pps[:, hs],
                                    wot[:, s, 128 * j:128 * (j + 1)],
                                    osb[:, s, hs], start=(s == 0),
                                    stop=(s == CS - 1))
                        nc.any.tensor_scalar(
                            out=fin[:, j, :], in0=pps[:],
                            scalar1=bot[:, j:j + 1], scalar2=None,
                            op0=mybir.AluOpType.add)
                        nc.any.tensor_tensor(out=fin[:, j, :], in0=fin[:, j, :],
                                             in1=xt[:, j, f, :],
                                             op=mybir.AluOpType.add)
                        nc.sync.dma_start(y[:, j, f, :], fin[:, j, :])

    nc.compile()
    return nc


class Runner:
    """Jitted SPMD executable for one built Bass program, reused across calls
    so the NEFF is loaded onto the devices only once."""

    def __init__(self, nc):
        bass2jax.install_neuronx_cc_hook()
        self.nc = nc
        pname = nc.partition_id_tensor.name if nc.partition_id_tensor else None
        in_names, out_names, out_avals = [], [], []
        for alloc in nc.m.functions[0].allocations:
            if not isinstance(alloc, mybir.MemoryLocationSet):
                continue
            name = alloc.memorylocations[0].name
            if alloc.kind == "ExternalInput":
                if name != pname:
                    in_names.append(name)
            elif alloc.kind == "ExternalOutput":
                out_names.append(name)
                out_avals.append(jax.core.ShapedArray(
                    tuple(alloc.tensor_shape), mybir.dt.np(alloc.dtype)))
        self.in_names, self.out_names, self.out_avals = \
            in_names, out_names, out_avals
        n_params = len(in_names)
        bind_names = in_names + out_names + ([pname] if pname else [])
        donate = tuple(range(n_params, n_params + len(out_names)))

        def _body(*args):
            operands = list(args)
            if pname:
                operands.append(bass2jax.partition_id_tensor())
            outs = bass2jax._bass_exec_p.bind(
                *operands, out_avals=tuple(out_avals),
                in_names=tuple(bind_names), out_names=tuple(out_names),
                lowering_input_output_aliases=(),
                sim_require_finite=True, sim_require_nnan=True, nc=nc)
            return tuple(outs)

        self.devices = jax.devices()[:NCORES]
        self.mesh = Mesh(np.asarray(self.devices), ("core",))
        nio = n_params + len(out_names)
        self.sharded = jax.jit(
            shard_map(_body, mesh=self.mesh,
                      in_specs=(PartitionSpec("core"),) * nio,
                      out_specs=(PartitionSpec("core"),) * len(out_names),
                      check_rep=False),
            donate_argnums=donate, keep_unused=True)

    def concat_inputs(self, in_maps):
        return [np.concatenate([np.asarray(m[n]) for m in in_maps], axis=0)
                for n in self.in_names]

    def fresh_zeros(self):
        return [np.zeros((NCORES * a.shape[0], *a.shape[1:]), a.dtype)
                for a in self.out_avals]

    def __call__(self, concat_in, zeros):
        out = self.sharded(*concat_in, *zeros)
        jax.block_until_ready(out)
        return out

    def run(self, in_maps):
        out = self(self.concat_inputs(in_maps), self.fresh_zeros())
        return [
            {n: np.asarray(out[i]).reshape(NCORES, *self.out_avals[i].shape)[c]
             for i, n in enumerate(self.out_names)}
            for c in range(NCORES)
        ]


def _get_runner(repeat: int = 1):
    if repeat not in _CACHE:
        _CACHE[repeat] = Runner(build_nc(repeat))
    return _CACHE[repeat]


def _prep_inputs(x, gamma, beta, wq, bq, wk, bk, wv, bv, wo, bo):
    """Host-side sharding / layout prep -> per-core input maps."""
    bf = ml_dtypes.bfloat16

    def wprep(w):
        # lhsT layout [ci, c_out] striped to [p, cs, c_out]
        return np.ascontiguousarray(
            w.T.reshape(CS, 128, C).transpose(1, 0, 2)).astype(bf)

    def vprep(v):
        # per-channel [C] -> [128, CS]
        return np.ascontiguousarray(v.reshape(CS, 128).T).astype(np.float32)

    wq_h, wk_h, wv_h, wo_h = wprep(wq), wprep(wk), wprep(wv), wprep(wo)
    bop = (wo.astype(np.float64) @ bv.astype(np.float64)).astype(np.float32) + bo
    shared = {
        "wq": wq_h, "wk": wk_h, "wv": wv_h, "wo": wo_h,
        "bq": vprep(bq), "bk": vprep(bk), "bop": vprep(bop),
        "gamma": vprep(gamma), "beta": vprep(beta),
    }

    frames = np.ascontiguousarray(
        x.transpose(0, 2, 1, 3, 4).reshape(F, C, N))  # [32, 256, 1024]
    in_maps = []
    for c in range(NCORES):
        sh = frames[FPC * c:FPC * (c + 1)]           # [4, 256, 1024]
        arr = np.ascontiguousarray(
            sh.transpose(1, 0, 2).reshape(CS, 128, FPC, N).transpose(1, 0, 2, 3))
        in_maps.append({"xin": arr.astype(np.float32), **shared})
    return in_maps


def _assemble(results):
    frames = np.empty((F, C, N), np.float32)
    for c in range(NCORES):
        arr = results[c]["y"]                        # [128, CS, FPC, N]
        frames[FPC * c:FPC * (c + 1)] = (
            arr.transpose(1, 0, 2, 3).reshape(C, FPC, N).transpose(1, 0, 2))
    return frames.reshape(B, T, C, H, W).transpose(0, 2, 1, 3, 4)


def kernel(**inputs):
    inputs = {k: np.asarray(v) for k, v in inputs.items()}
    in_maps = _prep_inputs(**inputs)
    runner = _get_runner()
    return _assemble(runner.run(in_maps))
